# revision 15
# baseline (speedup 1.0000x reference)
"""Distributed multi-head-attention kernel for 8 TRN2 NeuronCores.

Problem (hardcoded): B=4, T=S=1024, E=512, H=8, head_dim=64, fp32 I/O.
Sharding: core c handles batch b=c//2 and heads [4*(c%2), 4*(c%2)+4).
No collectives: each core produces a partial output projection
(contraction over its 256 ctx columns); the host sums the two partials
per batch and adds bo.

Compute dtype: bf16 on the TensorEngine (fp32 PSUM accumulation),
softmax in fp32 on ScalarE/VectorE.

v3 structure:
- Warm-up matmuls on a memset tile from t=0 keep the PE HAM un-throttled
  before the first DMA lands (v2 ran its first 25us at 1.2 GHz).
- k-outer projection loop consumes per-k-tile DMA chunks as they land.
- Scores for a head pair land in one 4-bank PSUM tile and one fused
  [128, 2048] ACTIVATE computes both heads' exp (ScalarE is the
  steady-state bottleneck: ~2.0us/iteration vs ~1.7us of PE work).
- Softmax-denominator tails run entirely in the endgame; normalize is
  all-bf16 and split across VectorE and GpSimdE.
- Output partials are stored bf16; the host sums in fp32.
"""

import numpy as np
import ml_dtypes

import concourse.bass as bass
import concourse.tile as tile
import concourse.mybir as mybir
from concourse.bass_utils import run_bass_kernel_spmd

BF16 = mybir.dt.bfloat16
F32 = mybir.dt.float32
NPBF16 = ml_dtypes.bfloat16

B, T, S, E = 4, 1024, 1024, 512
H, HD = 8, 64
N_CORES = 8
HPC = H // 2          # heads per core = 4
NP = HPC // 2         # head pairs per core = 2
OS = E // 2           # o-slice width per core = 256
KT = E // 128         # contraction k-tiles for projections = 4
TT = T // 128         # token tiles = 8
TC = T // 512         # 512-wide token chunks = 2
N_WARM = 10           # PE warm-up matmuls

# ---------------------------------------------------------------------------
# Walrus in this container rejects instructions carrying more than a couple of
# sync waits. After Tile scheduling, split excess waits onto same-engine NOPs
# inserted immediately before the over-subscribed instruction.
# ---------------------------------------------------------------------------
_MAX_WAITS = 1
_split_ctr = [0]


def _split_sync_waits(nc, max_waits=_MAX_WAITS):
    for f in nc.m.functions:
        for bb in f.blocks:
            insts = bb.instructions
            if not any(i.sync_info and i.sync_info.on_wait
                       and len(i.sync_info.on_wait) > max_waits for i in insts):
                continue
            new = []
            for inst in insts:
                si = inst.sync_info
                if si is not None and si.on_wait and len(si.on_wait) > max_waits:
                    waits = list(si.on_wait)
                    extra, keep = waits[:-max_waits], waits[-max_waits:]
                    for j in range(0, len(extra), max_waits):
                        _split_ctr[0] += 1
                        nop = mybir.InstNoOp(
                            name=f"syncsplit-{_split_ctr[0]}", ins=[], outs=[])
                        nop.engine = inst.engine
                        nop.bass_nofuse = True
                        nop.text_hint = "syncsplit"
                        nop.sync_info = mybir.SyncInfo(
                            on_wait=extra[j:j + max_waits], on_update=[])
                        new.append(nop)
                    si.on_wait = keep
                new.append(inst)
            bb.instructions = new


def _drain_and_barrier_light(self, tick_clock, wait_clock):
    from concourse.vector_clock import ScopedClock
    nc = self.nc
    drain_inst = nc.sync.drain()
    wait_clock.add_sem_waits(
        drain_inst.ins, ScopedClock({None: tick_clock.global_clock}))
    nc.all_engine_barrier()
    assert self.sems is not None
    popped = nc._tile_sem_poison_stack.pop()
    assert popped is self._sem_poison
    nc.clear_and_free_semaphores(list(self.sems.allocated().values()))


tile.TileContext._drain_and_barrier = _drain_and_barrier_light


# ---------------------------------------------------------------------------
# Kernel graph
# ---------------------------------------------------------------------------
def build_nc():
    nc = bass.Bass()

    # p-major layouts: [p, k, n] flattened so DMAs are contiguous per partition
    xqT = nc.declare_dram_parameter("xqT", [128, KT * T], BF16, isOutput=False)
    xkT = nc.declare_dram_parameter("xkT", [128, KT * S], BF16, isOutput=False)
    xvT = nc.declare_dram_parameter("xvT", [128, KT * S], BF16, isOutput=False)
    wqT = nc.declare_dram_parameter("wqT", [128, KT * OS], BF16, isOutput=False)
    wkT = nc.declare_dram_parameter("wkT", [128, KT * OS], BF16, isOutput=False)
    wvT = nc.declare_dram_parameter("wvT", [128, KT * OS], BF16, isOutput=False)
    # head-pair slices of Wo^T: [128 (= 2 heads x 64 c), 512 (e)] each
    woPs = [nc.declare_dram_parameter(f"woP{p}", [128, E], BF16, isOutput=False)
            for p in range(NP)]
    bq_t = nc.declare_dram_parameter("bq_t", [128, 2], F32, isOutput=False)
    bk_t = nc.declare_dram_parameter("bk_t", [128, 2], F32, isOutput=False)
    bv_b = nc.declare_dram_parameter("bv_b", [128, OS], F32, isOutput=False)

    ident = nc.declare_dram_parameter("ident", [128, 128], F32, isOutput=False)
    indic = nc.declare_dram_parameter("indic", [TT, TT * HD], BF16,
                                      isOutput=False)
    out_ext = nc.declare_dram_parameter("out", [T, E], BF16, isOutput=True)

    with tile.TileContext(nc) as tc:
        with (
            tc.tile_pool(name="inp", bufs=1) as inp,
            tc.tile_pool(name="wts", bufs=1) as wts,
            tc.tile_pool(name="act", bufs=1) as actp,
            tc.tile_pool(name="et", bufs=3) as etp,
            tc.tile_pool(name="rb", bufs=4) as rbp,
            tc.tile_pool(name="psum", bufs=1, space="PSUM") as psum,
        ):
            # ---- PE warm-up: memset a bf16 tile and run dummy matmuls so
            # the HAM clock gate opens (K=8/8) before real work lands.
            warm_sb = wts.tile([128, 512], BF16, name="warm")
            nc.gpsimd.memset(warm_sb[:], 0.0)
            warm_ps = psum.tile([128, KT, 512], F32, name="warmps", tag="A",
                                bufs=1)
            for i in range(N_WARM):
                nc.tensor.matmul(
                    warm_ps[:, i % KT, :], warm_sb[:, 0:128], warm_sb[:],
                    start=True, stop=True)

            # ---- input DMAs -------------------------------------------------
            # sync: xq/xk k-tile pairs in consumption order.
            # scalar: w q/k first (needed at ~2.5us), then biases.
            # gpsimd: v-path, then endgame-only tensors.
            xq_sb = inp.tile([128, KT, T], BF16)
            xk_sb = inp.tile([128, KT, S], BF16)
            xv_sb = inp.tile([128, KT, S], BF16)
            rrq = xqT.ap().rearrange("p (k t) -> p k t", k=KT)
            rrk = xkT.ap().rearrange("p (k t) -> p k t", k=KT)
            rrv = xvT.ap().rearrange("p (k t) -> p k t", k=KT)
            for k in range(KT):
                nc.sync.dma_start(xq_sb[:, k:k + 1, :], rrq[:, k:k + 1, :])
                nc.sync.dma_start(xk_sb[:, k:k + 1, :], rrk[:, k:k + 1, :])

            wq_sb = wts.tile([128, KT, OS], BF16)
            nc.scalar.dma_start(
                wq_sb[:], wqT.ap().rearrange("p (k o) -> p k o", k=KT))
            wk_sb = wts.tile([128, KT, OS], BF16)
            nc.scalar.dma_start(
                wk_sb[:], wkT.ap().rearrange("p (k o) -> p k o", k=KT))
            bq_sb = wts.tile([128, 2], F32, name="bq")
            nc.scalar.dma_start(bq_sb[:], bq_t.ap())
            bk_sb = wts.tile([128, 2], F32, name="bk")
            nc.scalar.dma_start(bk_sb[:], bk_t.ap())

            wv_sb = wts.tile([128, KT, OS], BF16)
            nc.gpsimd.dma_start(
                wv_sb[:], wvT.ap().rearrange("p (k o) -> p k o", k=KT))
            bv_sb = wts.tile([128, OS], F32, name="bv")
            nc.gpsimd.dma_start(bv_sb[:], bv_b.ap())
            for k in range(KT):
                nc.gpsimd.dma_start(xv_sb[:, k:k + 1, :], rrv[:, k:k + 1, :])
            wo_sbs = []
            for p in range(NP):
                wo_sb = wts.tile([128, E], BF16, name=f"wo{p}")
                nc.gpsimd.dma_start(wo_sb[:], woPs[p].ap())
                wo_sbs.append(wo_sb)
            id_sb = wts.tile([128, 128], F32, name="ident")
            nc.gpsimd.dma_start(id_sb[:], ident.ap())
            ind_sb = wts.tile([TT, TT * HD], BF16, name="ind")
            nc.gpsimd.dma_start(ind_sb[:], indic.ap())

            # ---- persistent activations ------------------------------------
            # q^T, k^T: [o(128) x t] tiles; o-tile p holds heads 2p, 2p+1.
            qT_sb = [actp.tile([128, T], BF16, name=f"qT{p}") for p in range(NP)]
            kT_sb = [actp.tile([128, S], BF16, name=f"kT{p}") for p in range(NP)]
            v_aug = [actp.tile([128, HPC, HD + 1], BF16, name=f"vaug{st}")
                     for st in range(TT)]
            # normalized ctx for a head pair: even head on partitions 0-63,
            # odd head on 64-127
            ctx_pair = [actp.tile([128, T], BF16, name=f"ctx{p}")
                        for p in range(NP)]

            # ---- projections: k-outer so each k-tile's matmuls run as soon
            # as its xq/xk chunk lands. q accumulates in tag-A quad (4 banks),
            # k in two tag-B doubles (4 banks).
            pq = psum.tile([128, KT, 512], F32, name="pq", tag="A", bufs=1)
            pk = [psum.tile([128, 2, 512], F32, name=f"pk{o}", tag="B", bufs=2)
                  for o in range(2)]
            for k in range(KT):
                for o in range(2):
                    for c in range(TC):
                        nc.tensor.matmul(
                            pq[:, 2 * o + c, :],
                            wq_sb[:, k, 128 * o:128 * (o + 1)],
                            xq_sb[:, k, 512 * c:512 * (c + 1)],
                            start=(k == 0), stop=(k == KT - 1),
                        )
                for o in range(2):
                    for c in range(TC):
                        nc.tensor.matmul(
                            pk[o][:, c, :],
                            wk_sb[:, k, 128 * o:128 * (o + 1)],
                            xk_sb[:, k, 512 * c:512 * (c + 1)],
                            start=(k == 0), stop=(k == KT - 1),
                        )
            for o in range(2):
                for c in range(TC):
                    nc.vector.tensor_scalar_add(
                        qT_sb[o][:, 512 * c:512 * (c + 1)],
                        pq[:, 2 * o + c, :], bq_sb[:, o:o + 1])
            for o in range(2):
                for c in range(TC):
                    nc.vector.tensor_scalar_add(
                        kT_sb[o][:, 512 * c:512 * (c + 1)],
                        pk[o][:, c, :], bk_sb[:, o:o + 1])

            # ---- score + exp helpers ---------------------------------------
            def scores_pair(p, st):
                # One [128, 2048] tile: even head cols 0-1023, odd 1024-2047.
                # Even head streams from qT/kT partitions 0-63, odd from
                # 64-127 (disjoint PE row-strips so LDWEIGHTS pulls ahead).
                s_ps = psum.tile([128, 2 * T], F32, name="sps", tag="A",
                                 bufs=1)
                for half in range(2):
                    po = HD * half
                    for c in range(TC):
                        nc.tensor.matmul(
                            s_ps[:, T * half + 512 * c:
                                 T * half + 512 * (c + 1)],
                            kT_sb[p][po:po + HD, 128 * st:128 * (st + 1)],
                            qT_sb[p][po:po + HD, 512 * c:512 * (c + 1)],
                            start=True, stop=True,
                        )
                return s_ps

            def exp_tile(s_ps):
                # One fused ACTIVATE for both heads (ScalarE is the
                # bottleneck: (2048+352)/1.2 = 2.0us per call).
                et = etp.tile([128, 2 * T], BF16, name="et")
                nc.scalar.activation(
                    et[:], s_ps[:], mybir.ActivationFunctionType.Exp,
                    scale=float(1.0 / np.sqrt(HD)))
                return et

            def v_proj(st):
                # v natural layout + a trailing ones column per head: the ctx
                # matmul then emits the softmax denominator on PSUM partition
                # 64 for free (matmul cost is N cycles regardless of M).
                nc.gpsimd.memset(v_aug[st][:, :, HD:HD + 1], 1.0)
                ps = psum.tile([128, OS], F32, name="projv", tag="B", bufs=2)
                for k in range(KT):
                    nc.tensor.matmul(
                        ps[:],
                        xv_sb[:, k, 128 * st:128 * (st + 1)],
                        wv_sb[:, k, :],
                        start=(k == 0), stop=(k == KT - 1),
                    )
                nc.vector.tensor_add(
                    v_aug[st][:, :, 0:HD],
                    ps.rearrange("p (h d) -> p h d", h=HPC),
                    bv_sb.rearrange("p (h d) -> p h d", h=HPC),
                )

            def ctx_mm(c_ps, h, st, et):
                half = h % 2
                for c in range(TC):
                    nc.tensor.matmul(
                        c_ps[:, 512 * c:512 * (c + 1)],
                        v_aug[st][:, h, :],
                        et[:, T * half + 512 * c:T * half + 512 * (c + 1)],
                        start=(st == 0), stop=(st == TT - 1),
                    )

            def tail_front(p, half, c_ps, eng):
                # Stage the denominator row (partition-shifted 64 -> 0) and
                # the 64 raw ctx rows out of PSUM so the banks free for the
                # next consumer.
                rsb = rbp.tile([1, T], F32, name="rsb", tag="rs", bufs=4)
                cst = rbp.tile([HD, T], F32, name="cstage", tag="cst", bufs=4)
                if eng is nc.scalar:
                    nc.scalar.copy(rsb[0:1, :], c_ps[HD:HD + 1, :])
                    nc.scalar.copy(cst[:], c_ps[0:HD, :])
                else:
                    nc.vector.tensor_copy(rsb[0:1, :], c_ps[HD:HD + 1, :])
                    nc.vector.tensor_copy(cst[:], c_ps[0:HD, :])
                return p, half, rsb, cst

            # ---- pipeline: scores(0,0) + exp, v_proj, then the 16-iteration
            # (pair, st) middle loop. Steady state: ScalarE exp (2.0us) gates
            # scores(i+1) (WAR on the single s_ps buffer) while ctx(i) runs.
            s_cur = scores_pair(0, 0)
            et_cur = exp_tile(s_cur)
            for st in range(TT):
                v_proj(st)

            fronts = {}
            c_e = c_o = None
            for i in range(1, 2 * TT + 1):
                p_prev, st_prev = divmod(i - 1, TT)
                if st_prev == 0:
                    c_e = psum.tile([HD + 1, T], F32, name="ctx_e", tag="B",
                                    bufs=2)
                    c_o = psum.tile([HD + 1, T], F32, name="ctx_o", tag="B",
                                    bufs=2)
                ctx_mm(c_e, 2 * p_prev, st_prev, et_cur)
                ctx_mm(c_o, 2 * p_prev + 1, st_prev, et_cur)
                if st_prev == TT - 1:
                    fronts[(p_prev, 0)] = tail_front(p_prev, 0, c_e, nc.vector)
                    fronts[(p_prev, 1)] = tail_front(p_prev, 1, c_o, nc.scalar)
                if i < 2 * TT:
                    p_i, st_i = divmod(i, TT)
                    s_nxt = scores_pair(p_i, st_i)
                    et_cur = exp_tile(s_nxt)

            # ---- endgame: 4 softmax-denominator tails (stage-interleaved),
            # all-bf16 normalize split DVE/GpSimd, then the output projection
            # in two 4-group PSUM quads with copies and stores chasing.
            tails = [fronts[(0, 0)], fronts[(0, 1)],
                     fronts[(1, 0)], fronts[(1, 1)]]

            # r [1, T] -> [128, TT] via skinny PE transposes (DVE reciprocal
            # is ~8 cycles/elem/lane: the free dim must be tiny).
            tps, rinvTs = [], []
            for t, (_, _, rsb, _) in enumerate(tails):
                tp = psum.tile([128, TT], F32, name="tp", tag="B", bufs=2)
                for c in range(TT):
                    nc.tensor.matmul(
                        tp[:, c:c + 1],
                        rsb[0:1, 128 * c:128 * (c + 1)],
                        id_sb[0:1, 0:1],
                        is_transpose=True, start=True, stop=True)
                tps.append(tp)
            for t in range(4):
                rinvT = rbp.tile([128, TT], F32, name="rinvT", tag="tmp",
                                 bufs=4)
                nc.vector.reciprocal(rinvT[:], tps[t][:])
                rinvTs.append(rinvT)
            # transpose back to [TT, 128] rows
            endA = psum.tile([128, 2048], F32, name="endA", tag="A", bufs=1)
            for t, rinvT in enumerate(rinvTs):
                nc.tensor.matmul(
                    endA[0:TT, 128 * t:128 * (t + 1)],
                    rinvT[:], id_sb[:],
                    is_transpose=True, start=True, stop=True)
            r8s = []
            for t in range(4):
                r8 = rbp.tile([TT, 128], BF16, name="r8", tag="tmp", bufs=4)
                src = endA[0:TT, 128 * t:128 * (t + 1)]
                if t % 2 == 0:
                    nc.vector.tensor_copy(r8[:], src)
                else:
                    nc.scalar.copy(r8[:], src)
                r8s.append(r8)
            # indicator-matmul broadcast of 1/r across the 64 ctx partitions,
            # staged to bf16 SBUF (ScalarE) so the normalize runs 2x on DVE /
            # GpSimd.
            for t in range(4):
                p_t, half, rsb, cst = tails[t]
                rb_ps = psum.tile([HD, T], F32, name="rbps", tag="B", bufs=2)
                for c in range(TT):
                    nc.tensor.matmul(
                        rb_ps[:, 128 * c:128 * (c + 1)],
                        ind_sb[:, HD * c:HD * (c + 1)],
                        r8s[t][:], start=True, stop=True)
                dst = ctx_pair[p_t][HD * half:HD * (half + 1), :]
                nc.vector.tensor_mul(dst, cst[:], rb_ps[:])

            # output projection: out[t, e] = sum_pair ctx_pair[:, t-tile]^T
            # @ woP. Four 2-group PSUM pairs rotating through tag B; copies
            # (scalar/vector alternate) and stores (sync/gpsimd alternate)
            # chase the matmuls.
            for q in range(4):
                o_ps = psum.tile([128, 2, E], F32, name="ops", tag="B",
                                 bufs=2)
                for j in range(2):
                    g = 2 * q + j
                    nc.tensor.matmul(
                        o_ps[:, j, :], ctx_pair[0][:, 128 * g:128 * (g + 1)],
                        wo_sbs[0][:], start=True, stop=False)
                    nc.tensor.matmul(
                        o_ps[:, j, :], ctx_pair[1][:, 128 * g:128 * (g + 1)],
                        wo_sbs[1][:], start=False, stop=True)
                for j in range(2):
                    g = 2 * q + j
                    o_sb = rbp.tile([128, E], BF16, name="osb", tag="tmp",
                                    bufs=4)
                    if g % 2 == 0:
                        nc.scalar.copy(o_sb[:], o_ps[:, j, :])
                    else:
                        nc.vector.tensor_copy(o_sb[:], o_ps[:, j, :])
                    (nc.sync if g % 2 == 0 else nc.gpsimd).dma_start(
                        out_ext.ap().rearrange("(g pp) e -> pp g e", pp=128)
                        [:, g:g + 1, :],
                        o_sb.rearrange("p (g e) -> p g e", g=1))

    _split_sync_waits(nc)
    return nc


_NC = None


def _get_nc():
    global _NC
    if _NC is None:
        _NC = build_nc()
    return _NC


# ---------------------------------------------------------------------------
# Host-side sharding / unsharding
# ---------------------------------------------------------------------------
def make_in_maps(queries, keys, values, Wq, bq, Wk, bk, Wv, bv, Wo):
    in_maps = []
    for c in range(N_CORES):
        b, hh = divmod(c, 2)
        osl = slice(OS * hh, OS * (hh + 1))
        bq_s = np.zeros((128, 2), np.float32)
        bq_s[:, 0] = bq[osl][0:128]
        bq_s[:, 1] = bq[osl][128:256]
        bk_s = np.zeros((128, 2), np.float32)
        bk_s[:, 0] = bk[osl][0:128]
        bk_s[:, 1] = bk[osl][128:256]

        def pmaj(a):
            # [E, N] -> [128, KT*N], k-tiles along the free axis
            e, n = a.shape
            return np.ascontiguousarray(
                a.reshape(KT, 128, n).transpose(1, 0, 2).reshape(128, KT * n))

        m = {
            "xqT": pmaj(queries[b].T).astype(NPBF16),
            "xkT": pmaj(keys[b].T).astype(NPBF16),
            "xvT": pmaj(values[b].T).astype(NPBF16),
            "wqT": pmaj(Wq[osl, :].T).astype(NPBF16),
            "wkT": pmaj(Wk[osl, :].T).astype(NPBF16),
            "wvT": pmaj(Wv[osl, :].T).astype(NPBF16),
            "bq_t": bq_s,
            "bk_t": bk_s,
            "bv_b": np.broadcast_to(
                bv[osl][None, :], (128, OS)).astype(np.float32).copy(),
            "ident": np.eye(128, dtype=np.float32),
            "indic": np.repeat(np.eye(TT), HD, axis=1).astype(NPBF16),
        }
        for p in range(NP):
            cs = slice(OS * hh + 128 * p, OS * hh + 128 * (p + 1))
            m[f"woP{p}"] = np.ascontiguousarray(Wo[:, cs].T).astype(NPBF16)
        in_maps.append(m)
    return in_maps


def run_device(in_maps, trace=False):
    nc = _get_nc()
    return run_bass_kernel_spmd(
        nc, in_maps, core_ids=list(range(N_CORES)), trace=trace)


def _numpy_reference(queries, keys, values, Wq, bq, Wk, bk, Wv, bv, Wo, bo,
                     q_padding_mask, key_padding_mask, attn_mask):
    q = queries @ Wq.T + bq
    k = keys @ Wk.T + bk
    v = values @ Wv.T + bv

    def split(x):
        b, l, e = x.shape
        return x.reshape(b, l, H, HD).transpose(0, 2, 1, 3)

    q, k, v = split(q), split(k), split(v)
    scores = np.einsum('bhtd,bhsd->bhts', q, k) / np.sqrt(HD)
    scores = np.where(key_padding_mask[:, None, None, :], -np.inf, scores)
    scores = np.where(~attn_mask[None, None, :, :], -np.inf, scores)
    scores = scores - scores.max(axis=-1, keepdims=True)
    w = np.exp(scores)
    w = w / w.sum(axis=-1, keepdims=True)
    w = np.where(q_padding_mask[:, None, :, None], 0.0, w)
    ctx = np.einsum('bhts,bhsd->bhtd', w, v)
    ctx = ctx.transpose(0, 2, 1, 3).reshape(queries.shape[0], -1, E)
    return (ctx @ Wo.T + bo).astype(np.float32)


def kernel(queries, keys, values, Wq, bq, Wk, bk, Wv, bv, Wo, bo,
           q_padding_mask, key_padding_mask, attn_mask):
    queries = np.asarray(queries, dtype=np.float32)
    keys = np.asarray(keys, dtype=np.float32)
    values = np.asarray(values, dtype=np.float32)
    Wq, bq = np.asarray(Wq, np.float32), np.asarray(bq, np.float32)
    Wk, bk = np.asarray(Wk, np.float32), np.asarray(bk, np.float32)
    Wv, bv = np.asarray(Wv, np.float32), np.asarray(bv, np.float32)
    Wo, bo = np.asarray(Wo, np.float32), np.asarray(bo, np.float32)
    q_padding_mask = np.asarray(q_padding_mask)
    key_padding_mask = np.asarray(key_padding_mask)
    attn_mask = np.asarray(attn_mask)

    # The device kernel skips masking (and softmax max-subtraction, valid for
    # this problem's bounded score range). Masks are all-trivial per the
    # problem spec; fall back to a host reference if they ever are not.
    if q_padding_mask.any() or key_padding_mask.any() or not attn_mask.all():
        return _numpy_reference(
            queries, keys, values, Wq, bq, Wk, bk, Wv, bv, Wo, bo,
            q_padding_mask, key_padding_mask, attn_mask)

    in_maps = make_in_maps(queries, keys, values, Wq, bq, Wk, bk, Wv, bv, Wo)
    res = run_device(in_maps, trace=False)
    out = np.empty((B, T, E), np.float32)
    for b in range(B):
        out[b] = (res.results[2 * b]["out"].astype(np.float32)
                  + res.results[2 * b + 1]["out"].astype(np.float32)
                  + bo[None, :])
    return out


# revision 20
# speedup vs baseline: 1.6078x; 1.6078x over previous
"""Distributed multi-head-attention kernel for 8 TRN2 NeuronCores.

Problem (hardcoded): B=4, T=S=1024, E=512, H=8, head_dim=64, fp32 I/O.
Sharding: core c handles batch b=c//2 and heads [4*(c%2), 4*(c%2)+4).
No collectives: each core produces a partial output projection
(contraction over its 256 ctx columns); the host sums the two partials
per batch and adds bo.

Compute dtype: bf16 on the TensorEngine (fp32 PSUM accumulation),
softmax in fp32 on ScalarE/VectorE.

v3 structure:
- Warm-up matmuls on a memset tile from t=0 keep the PE HAM un-throttled
  before the first DMA lands (v2 ran its first 25us at 1.2 GHz).
- k-outer projection loop consumes per-k-tile DMA chunks as they land.
- Scores for a head pair land in one 4-bank PSUM tile and one fused
  [128, 2048] ACTIVATE computes both heads' exp (ScalarE is the
  steady-state bottleneck: ~2.0us/iteration vs ~1.7us of PE work).
- Softmax-denominator tails run entirely in the endgame; normalize is
  all-bf16 and split across VectorE and GpSimdE.
- Output partials are stored bf16; the host sums in fp32.
"""

import numpy as np
import ml_dtypes

import concourse.bass as bass
import concourse.tile as tile
import concourse.mybir as mybir
from concourse.bass_utils import run_bass_kernel_spmd

BF16 = mybir.dt.bfloat16
F32 = mybir.dt.float32
NPBF16 = ml_dtypes.bfloat16

B, T, S, E = 4, 1024, 1024, 512
H, HD = 8, 64
N_CORES = 8
HPC = H // 2          # heads per core = 4
NP = HPC // 2         # head pairs per core = 2
OS = E // 2           # o-slice width per core = 256
KT = E // 128         # contraction k-tiles for projections = 4
TT = T // 128         # token tiles = 8
TC = T // 512         # 512-wide token chunks = 2
N_WARM = 10           # PE warm-up matmuls

# ---------------------------------------------------------------------------
# Walrus in this container rejects instructions carrying more than a couple of
# sync waits. After Tile scheduling, split excess waits onto same-engine NOPs
# inserted immediately before the over-subscribed instruction.
# ---------------------------------------------------------------------------
_MAX_WAITS = 1
_split_ctr = [0]


def _split_sync_waits(nc, max_waits=_MAX_WAITS):
    for f in nc.m.functions:
        for bb in f.blocks:
            insts = bb.instructions
            if not any(i.sync_info and i.sync_info.on_wait
                       and len(i.sync_info.on_wait) > max_waits for i in insts):
                continue
            new = []
            for inst in insts:
                si = inst.sync_info
                if si is not None and si.on_wait and len(si.on_wait) > max_waits:
                    waits = list(si.on_wait)
                    extra, keep = waits[:-max_waits], waits[-max_waits:]
                    for j in range(0, len(extra), max_waits):
                        _split_ctr[0] += 1
                        nop = mybir.InstNoOp(
                            name=f"syncsplit-{_split_ctr[0]}", ins=[], outs=[])
                        nop.engine = inst.engine
                        nop.bass_nofuse = True
                        nop.text_hint = "syncsplit"
                        nop.sync_info = mybir.SyncInfo(
                            on_wait=extra[j:j + max_waits], on_update=[])
                        new.append(nop)
                    si.on_wait = keep
                new.append(inst)
            bb.instructions = new


def _drain_and_barrier_light(self, tick_clock, wait_clock):
    from concourse.vector_clock import ScopedClock
    nc = self.nc
    drain_inst = nc.sync.drain()
    wait_clock.add_sem_waits(
        drain_inst.ins, ScopedClock({None: tick_clock.global_clock}))
    nc.all_engine_barrier()
    assert self.sems is not None
    popped = nc._tile_sem_poison_stack.pop()
    assert popped is self._sem_poison
    nc.clear_and_free_semaphores(list(self.sems.allocated().values()))


tile.TileContext._drain_and_barrier = _drain_and_barrier_light


# ---------------------------------------------------------------------------
# Kernel graph
# ---------------------------------------------------------------------------
def build_nc():
    nc = bass.Bass()

    # p-major layouts: [p, k, n] flattened so DMAs are contiguous per partition
    xqT = nc.declare_dram_parameter("xqT", [128, KT * T], BF16, isOutput=False)
    xkT = nc.declare_dram_parameter("xkT", [128, KT * S], BF16, isOutput=False)
    xvT = nc.declare_dram_parameter("xvT", [128, KT * S], BF16, isOutput=False)
    wqT = nc.declare_dram_parameter("wqT", [128, KT * OS], BF16, isOutput=False)
    wkT = nc.declare_dram_parameter("wkT", [128, KT * OS], BF16, isOutput=False)
    wvT = nc.declare_dram_parameter("wvT", [128, KT * OS], BF16, isOutput=False)
    # head-pair slices of Wo^T: [128 (= 2 heads x 64 c), 512 (e)] each
    woPs = [nc.declare_dram_parameter(f"woP{p}", [128, E], BF16, isOutput=False)
            for p in range(NP)]
    bq_t = nc.declare_dram_parameter("bq_t", [128, 2], F32, isOutput=False)
    bk_t = nc.declare_dram_parameter("bk_t", [128, 2], F32, isOutput=False)
    bv_b = nc.declare_dram_parameter("bv_b", [128, OS], F32, isOutput=False)

    ident = nc.declare_dram_parameter("ident", [128, 128], F32, isOutput=False)
    indic = nc.declare_dram_parameter("indic", [TT, TT * HD], BF16,
                                      isOutput=False)
    out_ext = nc.declare_dram_parameter("out", [T, E], BF16, isOutput=True)

    with tile.TileContext(nc) as tc:
        with (
            tc.tile_pool(name="inp", bufs=1) as inp,
            tc.tile_pool(name="wts", bufs=1) as wts,
            tc.tile_pool(name="act", bufs=1) as actp,
            tc.tile_pool(name="et", bufs=4) as etp,
            tc.tile_pool(name="rb", bufs=4) as rbp,
            tc.tile_pool(name="psum", bufs=1, space="PSUM") as psum,
        ):
            # ---- PE warm-up: memset a bf16 tile and run dummy matmuls so
            # the HAM clock gate opens (K=8/8) before real work lands.
            warm_sb = wts.tile([128, 512], BF16, name="warm")
            nc.gpsimd.memset(warm_sb[:], 0.0)
            warm_ps = psum.tile([128, 2, 512], F32, name="warmps", tag="B",
                                bufs=2)
            for i in range(N_WARM):
                nc.tensor.matmul(
                    warm_ps[:, i % 2, :], warm_sb[:, 0:128], warm_sb[:],
                    start=True, stop=True)

            # ---- input DMAs -------------------------------------------------
            # sync ring: xq. scalar ring: wq/wk then xk. gpsimd (SWDGE):
            # biases, v-path, then endgame-only tensors. All three rings
            # stream concurrently at ~300 GB/s aggregate.
            xq_sb = inp.tile([128, KT, T], BF16)
            xk_sb = inp.tile([128, KT, S], BF16)
            xv_sb = inp.tile([128, KT, S], BF16)
            rrq = xqT.ap().rearrange("p (k t) -> p k t", k=KT)
            rrk = xkT.ap().rearrange("p (k t) -> p k t", k=KT)
            rrv = xvT.ap().rearrange("p (k t) -> p k t", k=KT)
            for k in range(KT):
                nc.sync.dma_start(xq_sb[:, k:k + 1, :], rrq[:, k:k + 1, :])

            wq_sb = wts.tile([128, KT, OS], BF16)
            nc.scalar.dma_start(
                wq_sb[:], wqT.ap().rearrange("p (k o) -> p k o", k=KT))
            wk_sb = wts.tile([128, KT, OS], BF16)
            nc.scalar.dma_start(
                wk_sb[:], wkT.ap().rearrange("p (k o) -> p k o", k=KT))
            for k in range(KT):
                nc.scalar.dma_start(xk_sb[:, k:k + 1, :], rrk[:, k:k + 1, :])

            bq_sb = wts.tile([128, 2], F32, name="bq")
            nc.gpsimd.dma_start(bq_sb[:], bq_t.ap())
            bk_sb = wts.tile([128, 2], F32, name="bk")
            nc.gpsimd.dma_start(bk_sb[:], bk_t.ap())
            bv_sb = wts.tile([128, OS], F32, name="bv")
            nc.gpsimd.dma_start(bv_sb[:], bv_b.ap())
            wv_sb = wts.tile([128, KT, OS], BF16)
            nc.gpsimd.dma_start(
                wv_sb[:], wvT.ap().rearrange("p (k o) -> p k o", k=KT))
            for k in range(KT):
                nc.gpsimd.dma_start(xv_sb[:, k:k + 1, :], rrv[:, k:k + 1, :])
            wo_sbs = []
            for p in range(NP):
                wo_sb = wts.tile([128, E], BF16, name=f"wo{p}")
                nc.gpsimd.dma_start(wo_sb[:], woPs[p].ap())
                wo_sbs.append(wo_sb)
            id_sb = wts.tile([128, 128], F32, name="ident")
            nc.gpsimd.dma_start(id_sb[:], ident.ap())
            ind_sb = wts.tile([TT, TT * HD], BF16, name="ind")
            nc.gpsimd.dma_start(ind_sb[:], indic.ap())

            # ---- persistent activations ------------------------------------
            # q^T, k^T: [o(128) x t] tiles; o-tile p holds heads 2p, 2p+1.
            qT_sb = [actp.tile([128, T], BF16, name=f"qT{p}") for p in range(NP)]
            kT_sb = [actp.tile([128, S], BF16, name=f"kT{p}") for p in range(NP)]
            v_aug = [actp.tile([128, HPC, HD + 1], BF16, name=f"vaug{st}")
                     for st in range(TT)]
            # normalized ctx for a head pair: even head on partitions 0-63,
            # odd head on 64-127
            ctx_pair = [actp.tile([128, T], BF16, name=f"ctx{p}")
                        for p in range(NP)]

            # ---- projections: k-outer so each k-tile's matmuls run as soon
            # as its xq/xk chunk lands. q accumulates in the two tag-S
            # doubles, k in the two tag-B doubles.
            pq = [psum.tile([128, 2, 512], F32, name=f"pq{o}", tag="S",
                            bufs=2) for o in range(2)]
            pk = [psum.tile([128, 2, 512], F32, name=f"pk{o}", tag="B",
                            bufs=2) for o in range(2)]
            for k in range(KT):
                for o in range(2):
                    for c in range(TC):
                        nc.tensor.matmul(
                            pq[o][:, c, :],
                            wq_sb[:, k, 128 * o:128 * (o + 1)],
                            xq_sb[:, k, 512 * c:512 * (c + 1)],
                            start=(k == 0), stop=(k == KT - 1),
                        )
                for o in range(2):
                    for c in range(TC):
                        nc.tensor.matmul(
                            pk[o][:, c, :],
                            wk_sb[:, k, 128 * o:128 * (o + 1)],
                            xk_sb[:, k, 512 * c:512 * (c + 1)],
                            start=(k == 0), stop=(k == KT - 1),
                        )
            for o in range(2):
                for c in range(TC):
                    nc.vector.tensor_scalar_add(
                        qT_sb[o][:, 512 * c:512 * (c + 1)],
                        pq[o][:, c, :], bq_sb[:, o:o + 1])
            for o in range(2):
                for c in range(TC):
                    nc.vector.tensor_scalar_add(
                        kT_sb[o][:, 512 * c:512 * (c + 1)],
                        pk[o][:, c, :], bk_sb[:, o:o + 1])

            # ---- score + exp helpers ---------------------------------------
            # Per-head [128, T] score tiles rotate through the two tag-S
            # slots: the WAR on a slot is exactly "previous same-half exp
            # finished", which keeps scores one iteration ahead of exp.
            def scores_head(p, st, half):
                s_ps = psum.tile([128, T], F32, name="sps", tag="S", bufs=2)
                po = HD * half
                for c in range(TC):
                    nc.tensor.matmul(
                        s_ps[:, 512 * c:512 * (c + 1)],
                        kT_sb[p][po:po + HD, 128 * st:128 * (st + 1)],
                        qT_sb[p][po:po + HD, 512 * c:512 * (c + 1)],
                        start=True, stop=True,
                    )
                return s_ps

            def exp_tile(s_ps):
                et = etp.tile([128, T], BF16, name="et")
                nc.scalar.activation(
                    et[:], s_ps[:], mybir.ActivationFunctionType.Exp,
                    scale=float(1.0 / np.sqrt(HD)))
                return et

            def v_proj(st):
                # v natural layout + a trailing ones column per head: the ctx
                # matmul then emits the softmax denominator on PSUM partition
                # 64 for free (matmul cost is N cycles regardless of M).
                nc.gpsimd.memset(v_aug[st][:, :, HD:HD + 1], 1.0)
                ps = psum.tile([128, OS], F32, name="projv", tag="B", bufs=2)
                for k in range(KT):
                    nc.tensor.matmul(
                        ps[:],
                        xv_sb[:, k, 128 * st:128 * (st + 1)],
                        wv_sb[:, k, :],
                        start=(k == 0), stop=(k == KT - 1),
                    )
                nc.vector.tensor_add(
                    v_aug[st][:, :, 0:HD],
                    ps.rearrange("p (h d) -> p h d", h=HPC),
                    bv_sb.rearrange("p (h d) -> p h d", h=HPC),
                )

            def ctx_mm(c_ps, h, st, et):
                for c in range(TC):
                    nc.tensor.matmul(
                        c_ps[:, 512 * c:512 * (c + 1)],
                        v_aug[st][:, h, :],
                        et[:, 512 * c:512 * (c + 1)],
                        start=(st == 0), stop=(st == TT - 1),
                    )

            def tail_front(p, half, c_ps, eng):
                # Stage the denominator row (partition-shifted 64 -> 0) and
                # the 64 raw ctx rows out of PSUM so the banks free for the
                # next consumer.
                rsb = rbp.tile([1, T], F32, name="rsb", tag="rs", bufs=4)
                cst = rbp.tile([HD, T], F32, name="cstage", tag="cst", bufs=4)
                if eng is nc.scalar:
                    nc.scalar.copy(rsb[0:1, :], c_ps[HD:HD + 1, :])
                    nc.scalar.copy(cst[:], c_ps[0:HD, :])
                else:
                    nc.vector.tensor_copy(rsb[0:1, :], c_ps[HD:HD + 1, :])
                    nc.vector.tensor_copy(cst[:], c_ps[0:HD, :])
                return p, half, rsb, cst

            # ---- pipeline: scores/exp for iteration 0, v_proj, then the
            # 16-iteration (pair, st) middle loop. Steady state is
            # ScalarE-bound (2 x 1.15us exp per iteration) with PE filling
            # the exp time with next-iteration scores and current ctx.
            s_e = scores_head(0, 0, 0)
            s_o = scores_head(0, 0, 1)
            et_e, et_o = exp_tile(s_e), exp_tile(s_o)
            for st in range(TT):
                v_proj(st)

            fronts = {}
            c_e = c_o = None
            for i in range(1, 2 * TT + 1):
                p_prev, st_prev = divmod(i - 1, TT)
                # scores(i) first: at the pair boundary ctx(i-1) waits on the
                # tail-front copies, and scores must not sit behind it in the
                # PE queue.
                et_e_n = et_o_n = None
                if i < 2 * TT:
                    p_i, st_i = divmod(i, TT)
                    s_e_n = scores_head(p_i, st_i, 0)
                    et_e_n = exp_tile(s_e_n)
                    s_o_n = scores_head(p_i, st_i, 1)
                    et_o_n = exp_tile(s_o_n)
                if st_prev == 0:
                    c_e = psum.tile([HD + 1, T], F32, name="ctx_e", tag="B",
                                    bufs=2)
                    c_o = psum.tile([HD + 1, T], F32, name="ctx_o", tag="B",
                                    bufs=2)
                ctx_mm(c_e, 2 * p_prev, st_prev, et_e)
                ctx_mm(c_o, 2 * p_prev + 1, st_prev, et_o)
                if st_prev == TT - 1:
                    fronts[(p_prev, 0)] = tail_front(p_prev, 0, c_e, nc.vector)
                    fronts[(p_prev, 1)] = tail_front(p_prev, 1, c_o, nc.vector)
                et_e, et_o = et_e_n, et_o_n

            # ---- endgame: 4 softmax-denominator tails (stage-interleaved),
            # all-bf16 normalize split DVE/GpSimd, then the output projection
            # in two 4-group PSUM quads with copies and stores chasing.
            tails = [fronts[(0, 0)], fronts[(0, 1)],
                     fronts[(1, 0)], fronts[(1, 1)]]

            # r [1, T] -> [128, TT] via skinny PE transposes (DVE reciprocal
            # is ~8 cycles/elem/lane: the free dim must be tiny).
            tps, rinvTs = [], []
            for t, (_, _, rsb, _) in enumerate(tails):
                tp = psum.tile([128, TT], F32, name="tp", tag="S", bufs=2)
                for c in range(TT):
                    nc.tensor.matmul(
                        tp[:, c:c + 1],
                        rsb[0:1, 128 * c:128 * (c + 1)],
                        id_sb[0:1, 0:1],
                        is_transpose=True, start=True, stop=True)
                tps.append(tp)
            for t in range(4):
                rinvT = rbp.tile([128, TT], F32, name="rinvT", tag="tmp",
                                 bufs=4)
                nc.vector.reciprocal(rinvT[:], tps[t][:])
                rinvTs.append(rinvT)
            # transpose back to [TT, 128] rows
            tpbs = []
            for t, rinvT in enumerate(rinvTs):
                tpb = psum.tile([TT, 128], F32, name="tpb", tag="S", bufs=2)
                nc.tensor.matmul(
                    tpb[:], rinvT[:], id_sb[:],
                    is_transpose=True, start=True, stop=True)
                tpbs.append(tpb)
            r8s = []
            for t in range(4):
                r8 = rbp.tile([TT, 128], BF16, name="r8", tag="tmp", bufs=4)
                if t % 2 == 0:
                    nc.vector.tensor_copy(r8[:], tpbs[t][:])
                else:
                    nc.scalar.copy(r8[:], tpbs[t][:])
                r8s.append(r8)
            # indicator-matmul broadcast of 1/r across the 64 ctx partitions,
            # staged to bf16 SBUF (ScalarE) so the normalize runs 2x on DVE /
            # GpSimd.
            for t in range(4):
                p_t, half, rsb, cst = tails[t]
                rb_ps = psum.tile([HD, T], F32, name="rbps", tag="B", bufs=2)
                for c in range(TT):
                    nc.tensor.matmul(
                        rb_ps[:, 128 * c:128 * (c + 1)],
                        ind_sb[:, HD * c:HD * (c + 1)],
                        r8s[t][:], start=True, stop=True)
                dst = ctx_pair[p_t][HD * half:HD * (half + 1), :]
                nc.vector.tensor_mul(dst, cst[:], rb_ps[:])

            # output projection: out[t, e] = sum_pair ctx_pair[:, t-tile]^T
            # @ woP. Four 2-group PSUM pairs rotating through tag B; copies
            # (scalar/vector alternate) and stores (sync/gpsimd alternate)
            # chase the matmuls.
            for q in range(4):
                o_ps = psum.tile([128, 2, E], F32, name="ops", tag="B",
                                 bufs=2)
                for j in range(2):
                    g = 2 * q + j
                    nc.tensor.matmul(
                        o_ps[:, j, :], ctx_pair[0][:, 128 * g:128 * (g + 1)],
                        wo_sbs[0][:], start=True, stop=False)
                    nc.tensor.matmul(
                        o_ps[:, j, :], ctx_pair[1][:, 128 * g:128 * (g + 1)],
                        wo_sbs[1][:], start=False, stop=True)
                for j in range(2):
                    g = 2 * q + j
                    o_sb = rbp.tile([128, E], BF16, name="osb", tag="tmp",
                                    bufs=4)
                    if g % 2 == 0:
                        nc.scalar.copy(o_sb[:], o_ps[:, j, :])
                    else:
                        nc.vector.tensor_copy(o_sb[:], o_ps[:, j, :])
                    (nc.sync if g % 2 == 0 else nc.gpsimd).dma_start(
                        out_ext.ap().rearrange("(g pp) e -> pp g e", pp=128)
                        [:, g:g + 1, :],
                        o_sb.rearrange("p (g e) -> p g e", g=1))

    _split_sync_waits(nc)
    return nc


_NC = None


def _get_nc():
    global _NC
    if _NC is None:
        _NC = build_nc()
    return _NC


# ---------------------------------------------------------------------------
# Host-side sharding / unsharding
# ---------------------------------------------------------------------------
def make_in_maps(queries, keys, values, Wq, bq, Wk, bk, Wv, bv, Wo):
    in_maps = []
    for c in range(N_CORES):
        b, hh = divmod(c, 2)
        osl = slice(OS * hh, OS * (hh + 1))
        bq_s = np.zeros((128, 2), np.float32)
        bq_s[:, 0] = bq[osl][0:128]
        bq_s[:, 1] = bq[osl][128:256]
        bk_s = np.zeros((128, 2), np.float32)
        bk_s[:, 0] = bk[osl][0:128]
        bk_s[:, 1] = bk[osl][128:256]

        def pmaj(a):
            # [E, N] -> [128, KT*N], k-tiles along the free axis
            e, n = a.shape
            return np.ascontiguousarray(
                a.reshape(KT, 128, n).transpose(1, 0, 2).reshape(128, KT * n))

        m = {
            "xqT": pmaj(queries[b].T).astype(NPBF16),
            "xkT": pmaj(keys[b].T).astype(NPBF16),
            "xvT": pmaj(values[b].T).astype(NPBF16),
            "wqT": pmaj(Wq[osl, :].T).astype(NPBF16),
            "wkT": pmaj(Wk[osl, :].T).astype(NPBF16),
            "wvT": pmaj(Wv[osl, :].T).astype(NPBF16),
            "bq_t": bq_s,
            "bk_t": bk_s,
            "bv_b": np.broadcast_to(
                bv[osl][None, :], (128, OS)).astype(np.float32).copy(),
            "ident": np.eye(128, dtype=np.float32),
            "indic": np.repeat(np.eye(TT), HD, axis=1).astype(NPBF16),
        }
        for p in range(NP):
            cs = slice(OS * hh + 128 * p, OS * hh + 128 * (p + 1))
            m[f"woP{p}"] = np.ascontiguousarray(Wo[:, cs].T).astype(NPBF16)
        in_maps.append(m)
    return in_maps


def run_device(in_maps, trace=False):
    nc = _get_nc()
    return run_bass_kernel_spmd(
        nc, in_maps, core_ids=list(range(N_CORES)), trace=trace)


def _numpy_reference(queries, keys, values, Wq, bq, Wk, bk, Wv, bv, Wo, bo,
                     q_padding_mask, key_padding_mask, attn_mask):
    q = queries @ Wq.T + bq
    k = keys @ Wk.T + bk
    v = values @ Wv.T + bv

    def split(x):
        b, l, e = x.shape
        return x.reshape(b, l, H, HD).transpose(0, 2, 1, 3)

    q, k, v = split(q), split(k), split(v)
    scores = np.einsum('bhtd,bhsd->bhts', q, k) / np.sqrt(HD)
    scores = np.where(key_padding_mask[:, None, None, :], -np.inf, scores)
    scores = np.where(~attn_mask[None, None, :, :], -np.inf, scores)
    scores = scores - scores.max(axis=-1, keepdims=True)
    w = np.exp(scores)
    w = w / w.sum(axis=-1, keepdims=True)
    w = np.where(q_padding_mask[:, None, :, None], 0.0, w)
    ctx = np.einsum('bhts,bhsd->bhtd', w, v)
    ctx = ctx.transpose(0, 2, 1, 3).reshape(queries.shape[0], -1, E)
    return (ctx @ Wo.T + bo).astype(np.float32)


def kernel(queries, keys, values, Wq, bq, Wk, bk, Wv, bv, Wo, bo,
           q_padding_mask, key_padding_mask, attn_mask):
    queries = np.asarray(queries, dtype=np.float32)
    keys = np.asarray(keys, dtype=np.float32)
    values = np.asarray(values, dtype=np.float32)
    Wq, bq = np.asarray(Wq, np.float32), np.asarray(bq, np.float32)
    Wk, bk = np.asarray(Wk, np.float32), np.asarray(bk, np.float32)
    Wv, bv = np.asarray(Wv, np.float32), np.asarray(bv, np.float32)
    Wo, bo = np.asarray(Wo, np.float32), np.asarray(bo, np.float32)
    q_padding_mask = np.asarray(q_padding_mask)
    key_padding_mask = np.asarray(key_padding_mask)
    attn_mask = np.asarray(attn_mask)

    # The device kernel skips masking (and softmax max-subtraction, valid for
    # this problem's bounded score range). Masks are all-trivial per the
    # problem spec; fall back to a host reference if they ever are not.
    if q_padding_mask.any() or key_padding_mask.any() or not attn_mask.all():
        return _numpy_reference(
            queries, keys, values, Wq, bq, Wk, bk, Wv, bv, Wo, bo,
            q_padding_mask, key_padding_mask, attn_mask)

    in_maps = make_in_maps(queries, keys, values, Wq, bq, Wk, bk, Wv, bv, Wo)
    res = run_device(in_maps, trace=False)
    out = np.empty((B, T, E), np.float32)
    for b in range(B):
        out[b] = (res.results[2 * b]["out"].astype(np.float32)
                  + res.results[2 * b + 1]["out"].astype(np.float32)
                  + bo[None, :])
    return out


# revision 30
# speedup vs baseline: 1.6109x; 1.0020x over previous
"""Distributed multi-head-attention kernel for 8 TRN2 NeuronCores.

Problem (hardcoded): B=4, T=S=1024, E=512, H=8, head_dim=64, fp32 I/O.
Sharding: core c handles batch b=c//2 and heads [4*(c%2), 4*(c%2)+4).
No collectives: each core produces a partial output projection
(contraction over its 256 ctx columns); the host sums the two partials
per batch and adds bo.

Compute dtype: bf16 on the TensorEngine (fp32 PSUM accumulation),
softmax in fp32 on ScalarE/VectorE.

v3 structure:
- Warm-up matmuls on a memset tile from t=0 keep the PE HAM un-throttled
  before the first DMA lands (v2 ran its first 25us at 1.2 GHz).
- k-outer projection loop consumes per-k-tile DMA chunks as they land.
- Scores for a head pair land in one 4-bank PSUM tile and one fused
  [128, 2048] ACTIVATE computes both heads' exp (ScalarE is the
  steady-state bottleneck: ~2.0us/iteration vs ~1.7us of PE work).
- Softmax-denominator tails run entirely in the endgame; normalize is
  all-bf16 and split across VectorE and GpSimdE.
- Output partials are stored bf16; the host sums in fp32.
"""

import numpy as np
import ml_dtypes

import concourse.bass as bass
import concourse.tile as tile
import concourse.mybir as mybir
from concourse.bass_utils import run_bass_kernel_spmd

BF16 = mybir.dt.bfloat16
FP8 = mybir.dt.float8e4
F32 = mybir.dt.float32
NPBF16 = ml_dtypes.bfloat16
NPFP8 = ml_dtypes.float8_e4m3fn
# Host scales Wq/Wk/Wv (and their biases) by WSCALE so fp8e4m3 keeps
# mantissa bits on the ~0.02-sigma weights; the exp scale and a host-side
# 1/WSCALE on the output undo it exactly (powers of two).
WSCALE = 1.0

B, T, S, E = 4, 1024, 1024, 512
H, HD = 8, 64
N_CORES = 8
HPC = H // 2          # heads per core = 4
NP = HPC // 2         # head pairs per core = 2
OS = E // 2           # o-slice width per core = 256
KT = E // 128         # contraction k-tiles for projections = 4
TT = T // 128         # token tiles = 8
TC = T // 512         # 512-wide token chunks = 2
N_WARM = 10           # PE warm-up matmuls

# ---------------------------------------------------------------------------
# Walrus in this container rejects instructions carrying more than a couple of
# sync waits. After Tile scheduling, split excess waits onto same-engine NOPs
# inserted immediately before the over-subscribed instruction.
# ---------------------------------------------------------------------------
_MAX_WAITS = 1
_split_ctr = [0]


def _split_sync_waits(nc, max_waits=_MAX_WAITS):
    for f in nc.m.functions:
        for bb in f.blocks:
            insts = bb.instructions
            if not any(i.sync_info and i.sync_info.on_wait
                       and len(i.sync_info.on_wait) > max_waits for i in insts):
                continue
            new = []
            for inst in insts:
                si = inst.sync_info
                if si is not None and si.on_wait and len(si.on_wait) > max_waits:
                    waits = list(si.on_wait)
                    extra, keep = waits[:-max_waits], waits[-max_waits:]
                    for j in range(0, len(extra), max_waits):
                        _split_ctr[0] += 1
                        nop = mybir.InstNoOp(
                            name=f"syncsplit-{_split_ctr[0]}", ins=[], outs=[])
                        nop.engine = inst.engine
                        nop.bass_nofuse = True
                        nop.text_hint = "syncsplit"
                        nop.sync_info = mybir.SyncInfo(
                            on_wait=extra[j:j + max_waits], on_update=[])
                        new.append(nop)
                    si.on_wait = keep
                new.append(inst)
            bb.instructions = new


def _drain_and_barrier_light(self, tick_clock, wait_clock):
    from concourse.vector_clock import ScopedClock
    nc = self.nc
    drain_inst = nc.sync.drain()
    wait_clock.add_sem_waits(
        drain_inst.ins, ScopedClock({None: tick_clock.global_clock}))
    nc.all_engine_barrier()
    assert self.sems is not None
    popped = nc._tile_sem_poison_stack.pop()
    assert popped is self._sem_poison
    nc.clear_and_free_semaphores(list(self.sems.allocated().values()))


tile.TileContext._drain_and_barrier = _drain_and_barrier_light


# ---------------------------------------------------------------------------
# Kernel graph
# ---------------------------------------------------------------------------
def build_nc():
    nc = bass.Bass()

    # p-major layouts: [p, k, n] flattened so DMAs are contiguous per partition
    xqT = nc.declare_dram_parameter("xqT", [128, KT * T], BF16, isOutput=False)
    xkT = nc.declare_dram_parameter("xkT", [128, KT * S], BF16, isOutput=False)
    xvT = nc.declare_dram_parameter("xvT", [128, KT * S], BF16, isOutput=False)
    wqT = nc.declare_dram_parameter("wqT", [128, KT * OS], BF16, isOutput=False)
    wkT = nc.declare_dram_parameter("wkT", [128, KT * OS], BF16, isOutput=False)
    wvT = nc.declare_dram_parameter("wvT", [128, KT * OS], BF16, isOutput=False)
    # head-pair slices of Wo^T: [128 (= 2 heads x 64 c), 512 (e)] each
    woPs = [nc.declare_dram_parameter(f"woP{p}", [128, E], BF16, isOutput=False)
            for p in range(NP)]
    bq_t = nc.declare_dram_parameter("bq_t", [128, 2], F32, isOutput=False)
    bk_t = nc.declare_dram_parameter("bk_t", [128, 2], F32, isOutput=False)
    bv_b = nc.declare_dram_parameter("bv_b", [128, OS], F32, isOutput=False)

    ident = nc.declare_dram_parameter("ident", [128, 128], F32, isOutput=False)
    indic = nc.declare_dram_parameter("indic", [TT, TT * HD], BF16,
                                      isOutput=False)
    out_ext = nc.declare_dram_parameter("out", [T, E], BF16, isOutput=True)

    with tile.TileContext(nc) as tc:
        with (
            tc.tile_pool(name="inp", bufs=1) as inp,
            tc.tile_pool(name="wts", bufs=1) as wts,
            tc.tile_pool(name="act", bufs=1) as actp,
            tc.tile_pool(name="et", bufs=4) as etp,
            tc.tile_pool(name="rb", bufs=4) as rbp,
            tc.tile_pool(name="psum", bufs=1, space="PSUM") as psum,
        ):
            # ---- PE warm-up: memset a bf16 tile and run dummy matmuls so
            # the HAM clock gate opens (K=8/8) before real work lands.
            warm_sb = wts.tile([128, 512], BF16, name="warm")
            nc.gpsimd.memset(warm_sb[:], 0.0)
            warm_ps = psum.tile([128, 2, 512], F32, name="warmps", tag="B",
                                bufs=2)
            for i in range(N_WARM):
                nc.tensor.matmul(
                    warm_ps[:, i % 2, :], warm_sb[:, 0:128], warm_sb[:],
                    start=True, stop=True)

            # ---- input DMAs -------------------------------------------------
            # sync ring: xq. scalar ring: wq/wk then xk. gpsimd (SWDGE):
            # biases, v-path, then endgame-only tensors. All three rings
            # stream concurrently at ~300 GB/s aggregate.
            xq_sb = inp.tile([128, KT, T], BF16)
            xk_sb = inp.tile([128, KT, S], BF16)
            xv_sb = inp.tile([128, KT, S], BF16)
            rrq = xqT.ap().rearrange("p (k t) -> p k t", k=KT)
            rrk = xkT.ap().rearrange("p (k t) -> p k t", k=KT)
            rrv = xvT.ap().rearrange("p (k t) -> p k t", k=KT)
            for k in range(KT):
                nc.sync.dma_start(xq_sb[:, k:k + 1, :], rrq[:, k:k + 1, :])

            wq_sb = wts.tile([128, KT, OS], BF16)
            nc.scalar.dma_start(
                wq_sb[:], wqT.ap().rearrange("p (k o) -> p k o", k=KT))
            wk_sb = wts.tile([128, KT, OS], BF16)
            nc.scalar.dma_start(
                wk_sb[:], wkT.ap().rearrange("p (k o) -> p k o", k=KT))
            for k in range(KT):
                nc.scalar.dma_start(xk_sb[:, k:k + 1, :], rrk[:, k:k + 1, :])

            wv_sb = wts.tile([128, KT, OS], BF16)
            nc.gpsimd.dma_start(
                wv_sb[:], wvT.ap().rearrange("p (k o) -> p k o", k=KT))
            # xv transferred per st-tile (all KT k-chunks of one 128-token
            # slice), so v_proj(st) is gated on one early 128 KB transfer
            # instead of the whole 1 MB tensor.
            for st in range(TT):
                nc.gpsimd.dma_start(
                    xv_sb[:, :, 128 * st:128 * (st + 1)],
                    rrv[:, :, 128 * st:128 * (st + 1)])
            # tiny bias transfers ride the sync ring behind xq
            bq_sb = wts.tile([128, 2], F32, name="bq")
            nc.sync.dma_start(bq_sb[:], bq_t.ap())
            bk_sb = wts.tile([128, 2], F32, name="bk")
            nc.sync.dma_start(bk_sb[:], bk_t.ap())
            bv_sb = wts.tile([128, OS], F32, name="bv")
            nc.sync.dma_start(bv_sb[:], bv_b.ap())
            wo_sbs = []
            for p in range(NP):
                wo_sb = wts.tile([128, E], BF16, name=f"wo{p}")
                nc.gpsimd.dma_start(wo_sb[:], woPs[p].ap())
                wo_sbs.append(wo_sb)
            id_sb = wts.tile([128, 128], F32, name="ident")
            nc.gpsimd.dma_start(id_sb[:], ident.ap())
            ind_sb = wts.tile([TT, TT * HD], BF16, name="ind")
            nc.gpsimd.dma_start(ind_sb[:], indic.ap())

            # ---- persistent activations ------------------------------------
            # q^T, k^T: [o(128) x t] tiles; o-tile p holds heads 2p, 2p+1.
            qT_sb = [actp.tile([128, T], BF16, name=f"qT{p}") for p in range(NP)]
            kT_sb = [actp.tile([128, S], BF16, name=f"kT{p}") for p in range(NP)]
            v_aug = [actp.tile([128, HPC, HD + 1], BF16, name=f"vaug{st}")
                     for st in range(TT)]
            # normalized ctx for a head pair: even head on partitions 0-63,
            # odd head on 64-127
            ctx_pair = [actp.tile([128, T], BF16, name=f"ctx{p}")
                        for p in range(NP)]

            # ---- projections: k-outer so each k-tile's matmuls run as soon
            # as its xq/xk chunk lands. q accumulates in the two tag-S
            # doubles, k in the two tag-B doubles.
            pq = [psum.tile([128, 2, 512], F32, name=f"pq{o}", tag="S",
                            bufs=2) for o in range(2)]
            pk = [psum.tile([128, 2, 512], F32, name=f"pk{o}", tag="B",
                            bufs=2) for o in range(2)]
            for k in range(KT):
                for o in range(2):
                    for c in range(TC):
                        nc.tensor.matmul(
                            pq[o][:, c, :],
                            wq_sb[:, k, 128 * o:128 * (o + 1)],
                            xq_sb[:, k, 512 * c:512 * (c + 1)],
                            start=(k == 0), stop=(k == KT - 1),
                        )
                for o in range(2):
                    for c in range(TC):
                        nc.tensor.matmul(
                            pk[o][:, c, :],
                            wk_sb[:, k, 128 * o:128 * (o + 1)],
                            xk_sb[:, k, 512 * c:512 * (c + 1)],
                            start=(k == 0), stop=(k == KT - 1),
                        )
            for o in range(2):
                for c in range(TC):
                    nc.vector.tensor_scalar_add(
                        qT_sb[o][:, 512 * c:512 * (c + 1)],
                        pq[o][:, c, :], bq_sb[:, o:o + 1])
            for o in range(2):
                for c in range(TC):
                    nc.vector.tensor_scalar_add(
                        kT_sb[o][:, 512 * c:512 * (c + 1)],
                        pk[o][:, c, :], bk_sb[:, o:o + 1])

            # ---- score + exp helpers ---------------------------------------
            # Per-head [128, T] score tiles rotate through the two tag-S
            # slots: the WAR on a slot is exactly "previous same-half exp
            # finished", which keeps scores one iteration ahead of exp.
            def scores_head(p, st, half):
                s_ps = psum.tile([128, T], F32, name="sps", tag="S", bufs=2)
                po = HD * half
                for c in range(TC):
                    nc.tensor.matmul(
                        s_ps[:, 512 * c:512 * (c + 1)],
                        kT_sb[p][po:po + HD, 128 * st:128 * (st + 1)],
                        qT_sb[p][po:po + HD, 512 * c:512 * (c + 1)],
                        start=True, stop=True,
                    )
                return s_ps

            def exp_tile(s_ps):
                et = etp.tile([128, T], BF16, name="et")
                nc.scalar.activation(
                    et[:], s_ps[:], mybir.ActivationFunctionType.Exp,
                    scale=float(1.0 / (np.sqrt(HD) * WSCALE * WSCALE)))
                return et

            def v_proj(st):
                # v natural layout + a trailing ones column per head: the ctx
                # matmul then emits the softmax denominator on PSUM partition
                # 64 for free (matmul cost is N cycles regardless of M).
                nc.gpsimd.memset(v_aug[st][:, :, HD:HD + 1], 1.0)
                ps = psum.tile([128, OS], F32, name="projv", tag="B", bufs=2)
                for k in range(KT):
                    nc.tensor.matmul(
                        ps[:],
                        xv_sb[:, k, 128 * st:128 * (st + 1)],
                        wv_sb[:, k, :],
                        start=(k == 0), stop=(k == KT - 1),
                    )
                nc.vector.tensor_add(
                    v_aug[st][:, :, 0:HD],
                    ps.rearrange("p (h d) -> p h d", h=HPC),
                    bv_sb.rearrange("p (h d) -> p h d", h=HPC),
                )

            def ctx_mm(c_ps, h, st, et):
                for c in range(TC):
                    nc.tensor.matmul(
                        c_ps[:, 512 * c:512 * (c + 1)],
                        v_aug[st][:, h, :],
                        et[:, 512 * c:512 * (c + 1)],
                        start=(st == 0), stop=(st == TT - 1),
                    )

            def tail_front(p, half, c_ps, eng):
                # Stage the denominator row (partition-shifted 64 -> 0) and
                # the 64 raw ctx rows out of PSUM so the banks free for the
                # next consumer.
                rsb = rbp.tile([1, T], F32, name="rsb", tag="rs", bufs=4)
                cst = rbp.tile([HD, T], F32, name="cstage", tag="cst", bufs=4)
                if eng is nc.scalar:
                    nc.scalar.copy(rsb[0:1, :], c_ps[HD:HD + 1, :])
                    nc.scalar.copy(cst[:], c_ps[0:HD, :])
                else:
                    nc.vector.tensor_copy(rsb[0:1, :], c_ps[HD:HD + 1, :])
                    nc.vector.tensor_copy(cst[:], c_ps[0:HD, :])
                return p, half, rsb, cst

            # ---- pipeline: scores/exp for iteration 0, v_proj, then the
            # 16-iteration (pair, st) middle loop. Steady state is
            # ScalarE-bound (2 x 1.15us exp per iteration) with PE filling
            # the exp time with next-iteration scores and current ctx.
            s_e = scores_head(0, 0, 0)
            s_o = scores_head(0, 0, 1)
            et_e, et_o = exp_tile(s_e), exp_tile(s_o)
            for st in range(TT):
                v_proj(st)

            fronts = {}
            c_e = c_o = None
            for i in range(1, 2 * TT + 1):
                p_prev, st_prev = divmod(i - 1, TT)
                # scores(i) first: at the pair boundary ctx(i-1) waits on the
                # tail-front copies, and scores must not sit behind it in the
                # PE queue.
                et_e_n = et_o_n = None
                if i < 2 * TT:
                    p_i, st_i = divmod(i, TT)
                    s_e_n = scores_head(p_i, st_i, 0)
                    et_e_n = exp_tile(s_e_n)
                    s_o_n = scores_head(p_i, st_i, 1)
                    et_o_n = exp_tile(s_o_n)
                if st_prev == 0:
                    c_e = psum.tile([HD + 1, T], F32, name="ctx_e", tag="B",
                                    bufs=2)
                    c_o = psum.tile([HD + 1, T], F32, name="ctx_o", tag="B",
                                    bufs=2)
                ctx_mm(c_e, 2 * p_prev, st_prev, et_e)
                ctx_mm(c_o, 2 * p_prev + 1, st_prev, et_o)
                if st_prev == TT - 1:
                    # pair-0 fronts on DVE (ScalarE is exp-saturated
                    # mid-loop); pair-1 fronts on ScalarE (idle after the
                    # last exp, keeps DVE free for the normalize muls).
                    eng = nc.vector if p_prev == 0 else nc.scalar
                    fronts[(p_prev, 0)] = tail_front(p_prev, 0, c_e, eng)
                    fronts[(p_prev, 1)] = tail_front(p_prev, 1, c_o, eng)
                et_e, et_o = et_e_n, et_o_n

            # ---- endgame: 4 softmax-denominator tails (stage-interleaved),
            # all-bf16 normalize split DVE/GpSimd, then the output projection
            # in two 4-group PSUM quads with copies and stores chasing.
            tails = [fronts[(0, 0)], fronts[(0, 1)],
                     fronts[(1, 0)], fronts[(1, 1)]]

            # r [1, T] -> [128, TT] via skinny PE transposes (DVE reciprocal
            # is ~8 cycles/elem/lane: the free dim must be tiny).
            tps, rinvTs = [], []
            for t, (_, _, rsb, _) in enumerate(tails):
                tp = psum.tile([128, TT], F32, name="tp", tag="S", bufs=2)
                for c in range(TT):
                    nc.tensor.matmul(
                        tp[:, c:c + 1],
                        rsb[0:1, 128 * c:128 * (c + 1)],
                        id_sb[0:1, 0:1],
                        is_transpose=True, start=True, stop=True)
                tps.append(tp)
            for t in range(4):
                rinvT = rbp.tile([128, TT], F32, name="rinvT", tag="tmp",
                                 bufs=4)
                nc.vector.reciprocal(rinvT[:], tps[t][:])
                rinvTs.append(rinvT)
            # transpose back to [TT, 128] rows
            tpbs = []
            for t, rinvT in enumerate(rinvTs):
                tpb = psum.tile([TT, 128], F32, name="tpb", tag="S", bufs=2)
                nc.tensor.matmul(
                    tpb[:], rinvT[:], id_sb[:],
                    is_transpose=True, start=True, stop=True)
                tpbs.append(tpb)
            r8s = []
            for t in range(4):
                r8 = rbp.tile([TT, 128], BF16, name="r8", tag="tmp", bufs=4)
                nc.scalar.copy(r8[:], tpbs[t][:])
                r8s.append(r8)

            # indicator-matmul broadcast of 1/r across the 64 ctx partitions,
            # then the normalize muls: t0-t2 on DVE straight from PSUM, t3
            # staged to SBUF by ScalarE so GpSimd can run it concurrently
            # with t2.
            rbs = []
            for t in range(4):
                rb_ps = psum.tile([HD, T], F32, name="rbps", tag="B", bufs=2)
                for c in range(TT):
                    nc.tensor.matmul(
                        rb_ps[:, 128 * c:128 * (c + 1)],
                        ind_sb[:, HD * c:HD * (c + 1)],
                        r8s[t][:], start=True, stop=True)
                rbs.append(rb_ps)
                if t == 3:
                    rb_sb = rbp.tile([HD, T], F32, name="rbsb", tag="tmp",
                                     bufs=4)
                    nc.scalar.copy(rb_sb[:], rb_ps[:])
                p_t, half, rsb, cst = tails[t]
                dst = ctx_pair[p_t][HD * half:HD * (half + 1), :]
                if t == 3:
                    nc.gpsimd.tensor_mul(dst, cst[:], rb_sb[:])
                else:
                    nc.vector.tensor_mul(dst, cst[:], rb_ps[:])

            # output projection: out[t, e] = sum_pair ctx_pair[:, t-tile]^T
            # @ woP. Pair-0 start matmuls are issued for all 8 groups as soon
            # as pair-0's normalize lands (they only read ctx_pair[0]);
            # pair-1 stop matmuls, copies (scalar/vector alternate) and
            # stores (sync/gpsimd alternate) chase. ops pairs q0/q2 live in
            # the freed tag-S slots, q1/q3 in tag B.
            o_pss = []
            for q in range(4):
                o_ps = psum.tile([128, 2, E], F32, name="ops",
                                 tag=("S" if q % 2 == 0 else "B"), bufs=2)
                o_pss.append(o_ps)
                for j in range(2):
                    g = 2 * q + j
                    nc.tensor.matmul(
                        o_ps[:, j, :], ctx_pair[0][:, 128 * g:128 * (g + 1)],
                        wo_sbs[0][:], start=True, stop=False)
            for q in range(4):
                o_ps = o_pss[q]
                for j in range(2):
                    g = 2 * q + j
                    nc.tensor.matmul(
                        o_ps[:, j, :], ctx_pair[1][:, 128 * g:128 * (g + 1)],
                        wo_sbs[1][:], start=False, stop=True)
                for j in range(2):
                    g = 2 * q + j
                    o_sb = rbp.tile([128, E], BF16, name="osb", tag="tmp",
                                    bufs=4)
                    if g % 2 == 0:
                        nc.scalar.copy(o_sb[:], o_ps[:, j, :])
                    else:
                        nc.vector.tensor_copy(o_sb[:], o_ps[:, j, :])
                    (nc.sync if g % 2 == 0 else nc.gpsimd).dma_start(
                        out_ext.ap().rearrange("(g pp) e -> pp g e", pp=128)
                        [:, g:g + 1, :],
                        o_sb.rearrange("p (g e) -> p g e", g=1))

    _split_sync_waits(nc)
    return nc


_NC = None


def _get_nc():
    global _NC
    if _NC is None:
        _NC = build_nc()
    return _NC


# ---------------------------------------------------------------------------
# Host-side sharding / unsharding
# ---------------------------------------------------------------------------
def make_in_maps(queries, keys, values, Wq, bq, Wk, bk, Wv, bv, Wo):
    in_maps = []
    for c in range(N_CORES):
        b, hh = divmod(c, 2)
        osl = slice(OS * hh, OS * (hh + 1))
        bq_s = np.zeros((128, 2), np.float32)
        bq_s[:, 0] = bq[osl][0:128]
        bq_s[:, 1] = bq[osl][128:256]
        bk_s = np.zeros((128, 2), np.float32)
        bk_s[:, 0] = bk[osl][0:128]
        bk_s[:, 1] = bk[osl][128:256]

        def pmaj(a):
            # [E, N] -> [128, KT*N], k-tiles along the free axis
            e, n = a.shape
            return np.ascontiguousarray(
                a.reshape(KT, 128, n).transpose(1, 0, 2).reshape(128, KT * n))

        m = {
            "xqT": pmaj(queries[b].T).astype(NPBF16),
            "xkT": pmaj(keys[b].T).astype(NPBF16),
            "xvT": pmaj(values[b].T).astype(NPBF16),
            "wqT": pmaj(Wq[osl, :].T * WSCALE).astype(NPBF16),
            "wkT": pmaj(Wk[osl, :].T * WSCALE).astype(NPBF16),
            "wvT": pmaj(Wv[osl, :].T * WSCALE).astype(NPBF16),
            "bq_t": bq_s * WSCALE,
            "bk_t": bk_s * WSCALE,
            "bv_b": np.broadcast_to(
                bv[osl][None, :] * WSCALE, (128, OS)).astype(np.float32).copy(),
            "ident": np.eye(128, dtype=np.float32),
            "indic": np.repeat(np.eye(TT), HD, axis=1).astype(NPBF16),
        }
        for p in range(NP):
            cs = slice(OS * hh + 128 * p, OS * hh + 128 * (p + 1))
            m[f"woP{p}"] = np.ascontiguousarray(Wo[:, cs].T).astype(NPBF16)
        in_maps.append(m)
    return in_maps


def run_device(in_maps, trace=False):
    nc = _get_nc()
    return run_bass_kernel_spmd(
        nc, in_maps, core_ids=list(range(N_CORES)), trace=trace)


def _numpy_reference(queries, keys, values, Wq, bq, Wk, bk, Wv, bv, Wo, bo,
                     q_padding_mask, key_padding_mask, attn_mask):
    q = queries @ Wq.T + bq
    k = keys @ Wk.T + bk
    v = values @ Wv.T + bv

    def split(x):
        b, l, e = x.shape
        return x.reshape(b, l, H, HD).transpose(0, 2, 1, 3)

    q, k, v = split(q), split(k), split(v)
    scores = np.einsum('bhtd,bhsd->bhts', q, k) / np.sqrt(HD)
    scores = np.where(key_padding_mask[:, None, None, :], -np.inf, scores)
    scores = np.where(~attn_mask[None, None, :, :], -np.inf, scores)
    scores = scores - scores.max(axis=-1, keepdims=True)
    w = np.exp(scores)
    w = w / w.sum(axis=-1, keepdims=True)
    w = np.where(q_padding_mask[:, None, :, None], 0.0, w)
    ctx = np.einsum('bhts,bhsd->bhtd', w, v)
    ctx = ctx.transpose(0, 2, 1, 3).reshape(queries.shape[0], -1, E)
    return (ctx @ Wo.T + bo).astype(np.float32)


def kernel(queries, keys, values, Wq, bq, Wk, bk, Wv, bv, Wo, bo,
           q_padding_mask, key_padding_mask, attn_mask):
    queries = np.asarray(queries, dtype=np.float32)
    keys = np.asarray(keys, dtype=np.float32)
    values = np.asarray(values, dtype=np.float32)
    Wq, bq = np.asarray(Wq, np.float32), np.asarray(bq, np.float32)
    Wk, bk = np.asarray(Wk, np.float32), np.asarray(bk, np.float32)
    Wv, bv = np.asarray(Wv, np.float32), np.asarray(bv, np.float32)
    Wo, bo = np.asarray(Wo, np.float32), np.asarray(bo, np.float32)
    q_padding_mask = np.asarray(q_padding_mask)
    key_padding_mask = np.asarray(key_padding_mask)
    attn_mask = np.asarray(attn_mask)

    # The device kernel skips masking (and softmax max-subtraction, valid for
    # this problem's bounded score range). Masks are all-trivial per the
    # problem spec; fall back to a host reference if they ever are not.
    if q_padding_mask.any() or key_padding_mask.any() or not attn_mask.all():
        return _numpy_reference(
            queries, keys, values, Wq, bq, Wk, bk, Wv, bv, Wo, bo,
            q_padding_mask, key_padding_mask, attn_mask)

    in_maps = make_in_maps(queries, keys, values, Wq, bq, Wk, bk, Wv, bv, Wo)
    res = run_device(in_maps, trace=False)
    out = np.empty((B, T, E), np.float32)
    inv_ws = np.float32(1.0 / WSCALE)
    for b in range(B):
        out[b] = ((res.results[2 * b]["out"].astype(np.float32)
                   + res.results[2 * b + 1]["out"].astype(np.float32))
                  * inv_ws + bo[None, :])
    return out
